# revision 57
# baseline (speedup 1.0000x reference)
"""GSAT graph-attention kernel for 8 Trainium2 NeuronCores.

Math (per batch b):
  h = x @ W                                     [N, 512]
  ss[i] = h[i] . w_src ; sd[j] = h[j] . w_dst   (w_* = W @ a_* / H, folded)
  t[i,j] = (ss[i] + sd[j]) * adj[i,j] + gumbel(noise[b,i,j])
  A1 = softmax_j(t)
  A2 = softmax_j(A1)  -- linearized: exp(z) ~= a + b*z on z in [0,1], so
       A2 ~= (a + b*A1) / (a*N + b)   (row sums are exactly a*N + b)
  out[b,n] = sum_i A2[i,n] * k[i],  k = h @ W_out (folded before aggregation)

Sharding: 8 cores = (batch b in 0..3) x (row-half rb in 0..1).  Rows i are
sharded; softmax is along j (within-row), so each core computes its 2048
rows completely and produces a partial outT summed over its rows; host adds
the two row-half partials per batch.

Two block pipelines, mixed to balance ScalarE vs VectorE:
  A (ACT-heavy):  v=Ln(u+eps); g=Ln(-v+eps) [in-place];
                  sm=sd+ss (ts 4x); m=sm*adj (tt 2x); t=m-g (tt 2x);
                  e1=Exp(t) with accum rs1  [all in-place in one bf16 tile]
  B (DVE-heavy):  exp(t) = (1 + adj*(Es*Ed-1)) / (-v) with Es=exp(ss),
                  Ed=exp(sd):  v=Ln(u+eps) f32; q=recip_approx_fast(v)
                  [in-place]; P=Es*Ed-1 (ts 4x); M=P*adj (tt 2x);
                  e1=(M+1)*q (stt 1x, accum rs1)    [q<0; signs cancel via
                  the per-block rs1r folded into kt1]
Ln/Exp both live in the natural_log_exp_and_others ACT table set; the
get_activation_tables patch below stops bacc from bouncing between the
per-function sets (32 table loads -> 1).

PE aggregates in bf16: aggp[ns] += kt1^T e1, kt1 = k * rs1r * (b/(aN+b)).
The linear second softmax needs no second Exp pass; its constant term
a/(aN+b) * sum_i k[i,:] is added as a per-partition ACT bias in the epilogue.
"""

import os
import sys

for _p in ("/opt/trn_rl_repo",):
    if _p not in sys.path and os.path.isdir(_p):
        sys.path.insert(0, _p)

os.environ.setdefault("MYCRO_LOCAL_CACHE", "1")

import numpy as np
import ml_dtypes

B, N, IN_F, H, OUT_F = 4, 4096, 256, 8, 64
D = H * OUT_F          # 512
RB = N // 2            # 2048 rows per core
NBLK = RB // 128       # 16 row blocks per core
EPS = 1e-9
N_CORES = 8

# linear fit of exp(z) on [0,1] (optimized against the softmax-of-softmax)
LIN_A = 1.03
LIN_B = 1.546
RS2C = LIN_A * N + LIN_B

# blocks using the DVE-reciprocal pipeline (rest use the ACT pipeline)
B_BLOCKS = {9, 10, 12, 13, 14}

_cache = {}


def _build_module():
    import contextlib

    import concourse.bacc as bacc
    import concourse.tile as tile
    from concourse import mybir

    f32 = mybir.dt.float32
    bf16 = mybir.dt.bfloat16
    AF = mybir.ActivationFunctionType
    ALU = mybir.AluOpType

    # Make ln/exp resolve only to the set that holds both, so the table is
    # loaded once instead of per-activation.  Set names and order are kept
    # (the emitted act_func_set_id indexes the original list).
    orig_tables = bacc.get_activation_tables

    def _patched_tables(arch):
        out = {}
        for name, fns in orig_tables(arch).items():
            if name != "natural_log_exp_and_others":
                fns = fns - {AF.Ln, AF.Exp}
            out[name] = fns
        return out

    bacc.get_activation_tables = _patched_tables
    try:
        return _build_module_inner(bacc, tile, mybir, f32, bf16, AF, ALU)
    finally:
        bacc.get_activation_tables = orig_tables


def _build_module_inner(bacc, tile, mybir, f32, bf16, AF, ALU):
    import contextlib

    nc = bacc.Bacc("TRN2", target_bir_lowering=False)

    xT_d = nc.declare_dram_parameter("xT", [IN_F, N], bf16, isOutput=False)
    xTr_d = nc.declare_dram_parameter("xTr", [IN_F, RB], bf16, isOutput=False)
    adj_d = nc.declare_dram_parameter("adj_s", [RB, N], bf16, isOutput=False)
    nz_d = nc.declare_dram_parameter("noise_s", [RB, N], f32, isOutput=False)
    wsd_d = nc.declare_dram_parameter("wsd", [IN_F, 2], bf16, isOutput=False)
    Wko_d = nc.declare_dram_parameter("Wko", [IN_F, OUT_F], bf16, isOutput=False)
    outT_d = nc.declare_dram_parameter("outT", [OUT_F, N], f32, isOutput=True)

    with tile.TileContext(nc) as tc:
        with contextlib.ExitStack() as ctx:
            pers = ctx.enter_context(tc.tile_pool(name="pers", bufs=1))
            sdb = pers.tile([128, N], bf16)       # s_dst broadcast down partitions
            edb = pers.tile([128, N], bf16)       # exp(s_dst) broadcast
            ss_col = pers.tile([128, NBLK], f32)  # ss_col[p, ib] = s_src[row]
            es_col = pers.tile([128, NBLK], f32)  # exp(ss_col)
            ktil = [pers.tile([128, OUT_F], bf16, tag=f"k{ib}", name=f"k{ib}")
                    for ib in range(NBLK)]
            C64 = pers.tile([OUT_F, 1], f32)      # epilogue bias
            epsb = pers.tile([128, 1], f32)
            nc.vector.memset(epsb, EPS)

            spool = ctx.enter_context(tc.tile_pool(name="stream", bufs=3))
            wpool = ctx.enter_context(tc.tile_pool(name="work", bufs=2))
            rpool = ctx.enter_context(tc.tile_pool(name="smalls", bufs=4))

            # DMAs + first Ln; emitted for the first blocks BEFORE any other
            # ACT work so the in-order ACT queue starts on Ln immediately
            def stage_ln(ib):
                nz = spool.tile([128, N], f32, tag="nz", name=f"nz{ib}", bufs=3)
                if ib == NBLK - 1:
                    # last block: chunked DMA so the tail chain starts early
                    for c in range(4):
                        cs = slice(c * 1024, (c + 1) * 1024)
                        nc.sync.dma_start(out=nz[:, cs],
                                          in_=nz_d[ib * 128:(ib + 1) * 128, cs])
                else:
                    nc.sync.dma_start(out=nz, in_=nz_d[ib * 128:(ib + 1) * 128, :])
                ad = spool.tile([128, N], bf16, tag="ad", name=f"ad{ib}", bufs=3)
                nc.sync.dma_start(out=ad, in_=adj_d[ib * 128:(ib + 1) * 128, :])
                if ib in B_BLOCKS:
                    v = wpool.tile([128, N], f32, tag="vq", name=f"vq{ib}")
                    nc.scalar.activation(out=v, in_=nz, func=AF.Ln, bias=epsb,
                                         scale=1.0)
                elif ib == NBLK - 1:
                    v = wpool.tile([128, N], bf16, tag="v", name=f"v{ib}", bufs=3)
                    for c in range(4):
                        cs = slice(c * 1024, (c + 1) * 1024)
                        nc.scalar.activation(out=v[:, cs], in_=nz[:, cs],
                                             func=AF.Ln, bias=epsb, scale=1.0)
                else:
                    v = wpool.tile([128, N], bf16, tag="v", name=f"v{ib}", bufs=3)
                    nc.scalar.activation(out=v, in_=nz, func=AF.Ln, bias=epsb,
                                         scale=1.0)
                return ad, v

            # score-combination tile (DVE); separate stage so phase-0 DVE work
            # is not blocked behind it in queue order
            def stage_sm(ib):
                sm = wpool.tile([128, N], bf16, tag="sm", name=f"sm{ib}", bufs=3)
                if ib in B_BLOCKS:
                    nc.vector.tensor_scalar(out=sm, in0=edb,
                                            scalar1=es_col[:, ib:ib + 1],
                                            scalar2=-1.0,
                                            op0=ALU.mult, op1=ALU.add)
                else:
                    nc.vector.tensor_scalar(out=sm, in0=sdb,
                                            scalar1=ss_col[:, ib:ib + 1],
                                            scalar2=None, op0=ALU.add)
                return sm

            # ---------------- phase 0 ----------------
            with tc.tile_pool(name="p0", bufs=1) as p0, \
                 tc.tile_pool(name="ps0", bufs=1, space="PSUM") as ps0:
                xT2 = [p0.tile([128, N], bf16, tag=f"xT{fc}", name=f"xT{fc}")
                       for fc in range(2)]
                xTr2 = [p0.tile([128, RB], bf16, tag=f"xTr{fc}", name=f"xTr{fc}")
                        for fc in range(2)]
                wsdt = [p0.tile([128, 2], bf16, tag=f"wsd{fc}", name=f"wsdt{fc}")
                        for fc in range(2)]
                Wkot = [p0.tile([128, OUT_F], bf16, tag=f"Wko{fc}", name=f"Wkot{fc}")
                        for fc in range(2)]
                # DMA order: first block's tensors, then what the sd-broadcast
                # chain needs (xT, wsd), then the rest
                lns = {0: stage_ln(0)}
                for fc in range(2):
                    nc.sync.dma_start(out=xT2[fc], in_=xT_d[fc * 128:(fc + 1) * 128, :])
                    nc.sync.dma_start(out=wsdt[fc], in_=wsd_d[fc * 128:(fc + 1) * 128, :])
                lns[1] = stage_ln(1)
                for fc in range(2):
                    nc.sync.dma_start(out=xTr2[fc], in_=xTr_d[fc * 128:(fc + 1) * 128, :])
                    nc.sync.dma_start(out=Wkot[fc], in_=Wko_d[fc * 128:(fc + 1) * 128, :])
                lns[2] = stage_ln(2)

                # s_dst row [1, N] and exp of it; broadcast both down partitions
                sd_row = p0.tile([1, N], bf16)
                for jc in range(8):
                    sps = ps0.tile([1, 512], f32, tag="sps", bufs=2)
                    for fc in range(2):
                        nc.tensor.matmul(sps, wsdt[fc][:, 1:2],
                                         xT2[fc][:, jc * 512:(jc + 1) * 512],
                                         start=(fc == 0), stop=(fc == 1))
                    nc.vector.tensor_copy(sd_row[0:1, jc * 512:(jc + 1) * 512], sps)
                import concourse.bass as bass_mod

                def bcast(dst, row, nm):
                    scratch = nc.dram_tensor(nm, [1, N], bf16)
                    nc.sync.dma_start(out=scratch[:], in_=row)
                    ap = bass_mod.AP(tensor=scratch[:].tensor,
                                     offset=scratch[:].offset,
                                     ap=[[0, 128]] + list(scratch[:].ap)[1:])
                    nc.gpsimd.dma_start(out=dst, in_=ap)

                bcast(sdb, sd_row, "sd_scratch")
                # exp(sd) broadcast: one ACT pass over the broadcast tile
                nc.scalar.activation(out=edb, in_=sdb, func=AF.Exp)

                # ss_col[p, ib] = s_src of own row ib*128+p; es_col = exp
                sscol_ps = ps0.tile([128, NBLK], f32, tag="sscol")
                for ib in range(NBLK):
                    for fc in range(2):
                        nc.tensor.matmul(sscol_ps[:, ib:ib + 1],
                                         xTr2[fc][:, ib * 128:(ib + 1) * 128],
                                         wsdt[fc][:, 0:1],
                                         start=(fc == 0), stop=(fc == 1))
                nc.vector.tensor_copy(ss_col, sscol_ps)
                nc.scalar.activation(out=es_col, in_=ss_col, func=AF.Exp)

                # k = x @ (W @ W_out)  (Wko folded on host; no h needed)
                ones_col = p0.tile([128, 1], bf16)
                nc.vector.memset(ones_col, 1.0)
                ksum_ps = ps0.tile([OUT_F, 1], f32, tag="ksum")
                for ib in range(NBLK):
                    kps = ps0.tile([128, OUT_F], f32, tag="kps", bufs=2)
                    for fc in range(2):
                        nc.tensor.matmul(kps,
                                         xTr2[fc][:, ib * 128:(ib + 1) * 128],
                                         Wkot[fc],
                                         start=(fc == 0), stop=(fc == 1))
                    # fold b/(aN+b) into ktil so kt1 is a pure per-row scale
                    nc.vector.tensor_scalar(out=ktil[ib], in0=kps,
                                            scalar1=float(LIN_B / RS2C),
                                            scalar2=None, op0=ALU.mult)
                    # ksum[d] += sum_i k[i, d] over this block's rows
                    nc.tensor.matmul(ksum_ps, ktil[ib], ones_col,
                                     start=(ib == 0), stop=(ib == NBLK - 1))
                # ksum was built from the pre-scaled ktil; correct for it
                nc.vector.tensor_scalar(out=C64, in0=ksum_ps,
                                        scalar1=float(LIN_A / LIN_B), scalar2=None,
                                        op0=ALU.mult)

            # ---------------- main loop ----------------
            aggpool = ctx.enter_context(tc.tile_pool(name="agg", bufs=1,
                                                     space="PSUM"))
            aggp = [aggpool.tile([OUT_F, 512], f32, tag=f"agg{j}", name=f"agg{j}")
                    for j in range(8)]
            sms = {0: stage_sm(0), 1: stage_sm(1)}
            for ib in range(NBLK):
                if ib + 3 < NBLK:
                    lns[ib + 3] = stage_ln(ib + 3)
                if ib + 2 < NBLK:
                    sms[ib + 2] = stage_sm(ib + 2)
                ad, v = lns.pop(ib)
                sm = sms.pop(ib)
                last = ib == NBLK - 1
                if ib in B_BLOCKS:
                    rs1 = rpool.tile([128, 1], f32, tag="rs1")
                    # q = 1/v (negative); M = P*adj; e1 = (M+1)*q, rowsum
                    nc.vector.reciprocal_approx_fast(out=v, in_=v)
                    nc.vector.tensor_tensor(out=sm, in0=sm, in1=ad, op=ALU.mult)
                    nc.vector.scalar_tensor_tensor(out=sm, in0=sm, scalar=1.0,
                                                   in1=v, op0=ALU.add,
                                                   op1=ALU.mult, accum_out=rs1)
                elif not last:
                    rs1 = rpool.tile([128, 1], f32, tag="rs1")
                    # g = ln(-v+eps); m = sm*adj; t = m-g; e1 = exp(t), rowsum
                    nc.scalar.activation(out=v, in_=v, func=AF.Ln, bias=epsb,
                                         scale=-1.0)
                    nc.vector.tensor_tensor(out=sm, in0=sm, in1=ad, op=ALU.mult)
                    nc.vector.tensor_tensor(out=sm, in0=sm, in1=v,
                                            op=ALU.subtract)
                    nc.scalar.activation(out=sm, in_=sm, func=AF.Exp,
                                         accum_out=rs1)
                else:
                    # final block: run the chain in 4 column chunks so the
                    # post-DMA tail is short
                    rc = [rpool.tile([128, 1], f32, tag=f"rc{c}",
                                     name=f"rc{c}") for c in range(4)]
                    for c in range(4):
                        cs = slice(c * 1024, (c + 1) * 1024)
                        nc.scalar.activation(out=v[:, cs], in_=v[:, cs],
                                             func=AF.Ln, bias=epsb, scale=-1.0)
                        nc.vector.tensor_tensor(out=sm[:, cs], in0=sm[:, cs],
                                                in1=ad[:, cs], op=ALU.mult)
                        nc.vector.tensor_tensor(out=sm[:, cs], in0=sm[:, cs],
                                                in1=v[:, cs], op=ALU.subtract)
                        nc.scalar.activation(out=sm[:, cs], in_=sm[:, cs],
                                             func=AF.Exp, accum_out=rc[c])
                    rs1 = rpool.tile([128, 1], f32, tag="rs1")
                    nc.vector.tensor_tensor(out=rc[0], in0=rc[0], in1=rc[1],
                                            op=ALU.add)
                    nc.vector.tensor_tensor(out=rc[2], in0=rc[2], in1=rc[3],
                                            op=ALU.add)
                    nc.vector.tensor_tensor(out=rs1, in0=rc[0], in1=rc[2],
                                            op=ALU.add)

                rs1r = rpool.tile([128, 1], f32, tag="rs1r")
                nc.vector.reciprocal(rs1r, rs1)
                kt1 = rpool.tile([128, OUT_F], bf16, tag="kt1")
                nc.vector.tensor_scalar(out=kt1, in0=ktil[ib], scalar1=rs1r,
                                        scalar2=None, op0=ALU.mult)

                for ns in range(8):
                    nc.tensor.matmul(aggp[ns], kt1,
                                     sm[:, ns * 512:(ns + 1) * 512],
                                     start=(ib == 0), stop=(ib == NBLK - 1))

            # ---------------- epilogue ----------------
            with tc.tile_pool(name="fin", bufs=1) as fpool:
                outT = fpool.tile([OUT_F, N], f32)
                for ns in range(8):
                    if ns % 2 == 0:
                        nc.scalar.activation(out=outT[:, ns * 512:(ns + 1) * 512],
                                             in_=aggp[ns], func=AF.Identity,
                                             bias=C64, scale=1.0)
                    else:
                        nc.vector.tensor_scalar(out=outT[:, ns * 512:(ns + 1) * 512],
                                                in0=aggp[ns], scalar1=C64,
                                                scalar2=None, op0=ALU.add)
                    if ns % 2 == 1:
                        nc.sync.dma_start(
                            out=outT_d[:, (ns - 1) * 512:(ns + 1) * 512],
                            in_=outT[:, (ns - 1) * 512:(ns + 1) * 512])

    nc.compile()
    return nc


def _get_module():
    if "nc" not in _cache:
        _cache["nc"] = _build_module()
    return _cache["nc"]


def kernel(x, adj, noise, W, a_src, a_dst, W_out):
    from concourse.bass_utils import run_bass_kernel_spmd

    nc = _get_module()

    x = np.asarray(x, dtype=np.float32)
    adj = np.asarray(adj, dtype=np.float32)
    noise = np.asarray(noise, dtype=np.float32)
    W = np.asarray(W, dtype=np.float32)
    a_src = np.asarray(a_src, dtype=np.float32)
    a_dst = np.asarray(a_dst, dtype=np.float32)
    W_out = np.asarray(W_out, dtype=np.float32)

    # fold the per-head score weights: s = (x @ W) @ a_flat / H == x @ (W @ a_flat / H)
    w_src = (W @ a_src.reshape(-1)) / H
    w_dst = (W @ a_dst.reshape(-1)) / H
    wsd = np.ascontiguousarray(np.stack([w_src, w_dst], axis=1)).astype(ml_dtypes.bfloat16)
    adj_bf = adj.astype(ml_dtypes.bfloat16)  # exact for 0/1 values
    Wko = np.ascontiguousarray(W @ W_out).astype(ml_dtypes.bfloat16)

    in_maps = []
    for core in range(N_CORES):
        b, rb = core // 2, core % 2
        rows = slice(rb * RB, (rb + 1) * RB)
        xTb = np.ascontiguousarray(x[b].T).astype(ml_dtypes.bfloat16)
        in_maps.append({
            "xT": xTb,
            "xTr": np.ascontiguousarray(xTb[:, rows]),
            "adj_s": np.ascontiguousarray(adj_bf[rows, :]),
            "noise_s": np.ascontiguousarray(noise[b, rows, :]),
            "wsd": wsd,
            "Wko": Wko,
        })

    res = run_bass_kernel_spmd(nc, in_maps, list(range(N_CORES)))
    kernel._last_results = res

    out = np.empty((B, N, OUT_F), dtype=np.float32)
    for b in range(B):
        acc = res.results[2 * b]["outT"].astype(np.float32) + \
            res.results[2 * b + 1]["outT"].astype(np.float32)
        out[b] = acc.T
    return out


# revision 58
# speedup vs baseline: 1.1256x; 1.1256x over previous
"""GSAT graph-attention kernel for 8 Trainium2 NeuronCores.

Math (per batch b):
  h = x @ W                                     [N, 512]
  ss[i] = h[i] . w_src ; sd[j] = h[j] . w_dst   (w_* = W @ a_* / H, folded)
  t[i,j] = (ss[i] + sd[j]) * adj[i,j] + gumbel(noise[b,i,j])
  A1 = softmax_j(t)
  A2 = softmax_j(A1)  -- linearized: exp(z) ~= a + b*z on z in [0,1], so
       A2 ~= (a + b*A1) / (a*N + b)   (row sums are exactly a*N + b)
  out[b,n] = sum_i A2[i,n] * k[i],  k = h @ W_out (folded before aggregation)

Sharding: 8 cores = (batch b in 0..3) x (row-half rb in 0..1).  Rows i are
sharded; softmax is along j (within-row), so each core computes its 2048
rows completely and produces a partial outT summed over its rows; host adds
the two row-half partials per batch.

Two block pipelines, mixed to balance ScalarE vs VectorE:
  A (ACT-heavy):  v=Ln(u+eps); g=Ln(-v+eps) [in-place];
                  sm=sd+ss (ts 4x); m=sm*adj (tt 2x); t=m-g (tt 2x);
                  e1=Exp(t) with accum rs1  [all in-place in one bf16 tile]
  B (DVE-heavy):  exp(t) = (1 + adj*(Es*Ed-1)) / (-v) with Es=exp(ss),
                  Ed=exp(sd):  v=Ln(u+eps) f32; q=recip_approx_fast(v)
                  [in-place]; P=Es*Ed-1 (ts 4x); M=P*adj (tt 2x);
                  e1=(M+1)*q (stt 1x, accum rs1)    [q<0; signs cancel via
                  the per-block rs1r folded into kt1]
Ln/Exp both live in the natural_log_exp_and_others ACT table set; the
get_activation_tables patch below stops bacc from bouncing between the
per-function sets (32 table loads -> 1).

PE aggregates in bf16: aggp[ns] += kt1^T e1, kt1 = k * rs1r * (b/(aN+b)).
The linear second softmax needs no second Exp pass; its constant term
a/(aN+b) * sum_i k[i,:] is added as a per-partition ACT bias in the epilogue.
"""

import os
import sys

for _p in ("/opt/trn_rl_repo",):
    if _p not in sys.path and os.path.isdir(_p):
        sys.path.insert(0, _p)

os.environ.setdefault("MYCRO_LOCAL_CACHE", "1")

import numpy as np
import ml_dtypes

B, N, IN_F, H, OUT_F = 4, 4096, 256, 8, 64
D = H * OUT_F          # 512
RB = N // 2            # 2048 rows per core
NBLK = RB // 128       # 16 row blocks per core
EPS = 1e-9
N_CORES = 8

# linear fit of exp(z) on [0,1] (optimized against the softmax-of-softmax)
LIN_A = 1.03
LIN_B = 1.546
RS2C = LIN_A * N + LIN_B

# blocks using the DVE-reciprocal pipeline (rest use the ACT pipeline)
B_BLOCKS = {2, 5, 8, 11, 14}

_cache = {}


def _build_module():
    import contextlib

    import concourse.bacc as bacc
    import concourse.tile as tile
    from concourse import mybir

    f32 = mybir.dt.float32
    bf16 = mybir.dt.bfloat16
    AF = mybir.ActivationFunctionType
    ALU = mybir.AluOpType

    # Make ln/exp resolve only to the set that holds both, so the table is
    # loaded once instead of per-activation.  Set names and order are kept
    # (the emitted act_func_set_id indexes the original list).
    orig_tables = bacc.get_activation_tables

    def _patched_tables(arch):
        out = {}
        for name, fns in orig_tables(arch).items():
            if name != "natural_log_exp_and_others":
                fns = fns - {AF.Ln, AF.Exp}
            out[name] = fns
        return out

    bacc.get_activation_tables = _patched_tables
    try:
        return _build_module_inner(bacc, tile, mybir, f32, bf16, AF, ALU)
    finally:
        bacc.get_activation_tables = orig_tables


def _build_module_inner(bacc, tile, mybir, f32, bf16, AF, ALU):
    import contextlib

    nc = bacc.Bacc("TRN2", target_bir_lowering=False)

    xT_d = nc.declare_dram_parameter("xT", [IN_F, N], bf16, isOutput=False)
    xTr_d = nc.declare_dram_parameter("xTr", [IN_F, RB], bf16, isOutput=False)
    adj_d = nc.declare_dram_parameter("adj_s", [RB, N], bf16, isOutput=False)
    nz_d = nc.declare_dram_parameter("noise_s", [RB, N], f32, isOutput=False)
    wsd_d = nc.declare_dram_parameter("wsd", [IN_F, 2], bf16, isOutput=False)
    Wko_d = nc.declare_dram_parameter("Wko", [IN_F, OUT_F], bf16, isOutput=False)
    outT_d = nc.declare_dram_parameter("outT", [OUT_F, N], f32, isOutput=True)

    with tile.TileContext(nc) as tc:
        with contextlib.ExitStack() as ctx:
            pers = ctx.enter_context(tc.tile_pool(name="pers", bufs=1))
            sdb = pers.tile([128, N], bf16)       # s_dst broadcast down partitions
            edb = pers.tile([128, N], bf16)       # exp(s_dst) broadcast
            ss_col = pers.tile([128, NBLK], f32)  # ss_col[p, ib] = s_src[row]
            es_col = pers.tile([128, NBLK], f32)  # exp(ss_col)
            ktil = [pers.tile([128, OUT_F], bf16, tag=f"k{ib}", name=f"k{ib}")
                    for ib in range(NBLK)]
            C64 = pers.tile([OUT_F, 1], f32)      # epilogue bias
            epsb = pers.tile([128, 1], f32)
            nc.vector.memset(epsb, EPS)

            spool = ctx.enter_context(tc.tile_pool(name="stream", bufs=3))
            wpool = ctx.enter_context(tc.tile_pool(name="work", bufs=2))
            rpool = ctx.enter_context(tc.tile_pool(name="smalls", bufs=4))

            # DMAs + first Ln; emitted for the first blocks BEFORE any other
            # ACT work so the in-order ACT queue starts on Ln immediately
            def stage_ln(ib):
                nz = spool.tile([128, N], f32, tag="nz", name=f"nz{ib}", bufs=3)
                if ib == NBLK - 1:
                    # last block: chunked DMA so the tail chain starts early
                    for c in range(4):
                        cs = slice(c * 1024, (c + 1) * 1024)
                        nc.sync.dma_start(out=nz[:, cs],
                                          in_=nz_d[ib * 128:(ib + 1) * 128, cs])
                else:
                    nc.sync.dma_start(out=nz, in_=nz_d[ib * 128:(ib + 1) * 128, :])
                ad = spool.tile([128, N], bf16, tag="ad", name=f"ad{ib}", bufs=3)
                nc.sync.dma_start(out=ad, in_=adj_d[ib * 128:(ib + 1) * 128, :])
                if ib in B_BLOCKS:
                    v = wpool.tile([128, N], f32, tag="vq", name=f"vq{ib}")
                    nc.scalar.activation(out=v, in_=nz, func=AF.Ln, bias=epsb,
                                         scale=1.0)
                elif ib == NBLK - 1:
                    v = wpool.tile([128, N], bf16, tag="v", name=f"v{ib}", bufs=3)
                    for c in range(4):
                        cs = slice(c * 1024, (c + 1) * 1024)
                        nc.scalar.activation(out=v[:, cs], in_=nz[:, cs],
                                             func=AF.Ln, bias=epsb, scale=1.0)
                else:
                    v = wpool.tile([128, N], bf16, tag="v", name=f"v{ib}", bufs=3)
                    nc.scalar.activation(out=v, in_=nz, func=AF.Ln, bias=epsb,
                                         scale=1.0)
                return ad, v

            # score-combination tile (DVE); separate stage so phase-0 DVE work
            # is not blocked behind it in queue order
            def stage_sm(ib):
                sm = wpool.tile([128, N], bf16, tag="sm", name=f"sm{ib}", bufs=3)
                if ib in B_BLOCKS:
                    nc.vector.tensor_scalar(out=sm, in0=edb,
                                            scalar1=es_col[:, ib:ib + 1],
                                            scalar2=-1.0,
                                            op0=ALU.mult, op1=ALU.add)
                else:
                    nc.vector.tensor_scalar(out=sm, in0=sdb,
                                            scalar1=ss_col[:, ib:ib + 1],
                                            scalar2=None, op0=ALU.add)
                return sm

            # ---------------- phase 0 ----------------
            with tc.tile_pool(name="p0", bufs=1) as p0, \
                 tc.tile_pool(name="ps0", bufs=1, space="PSUM") as ps0:
                xT2 = [p0.tile([128, N], bf16, tag=f"xT{fc}", name=f"xT{fc}")
                       for fc in range(2)]
                xTr2 = [p0.tile([128, RB], bf16, tag=f"xTr{fc}", name=f"xTr{fc}")
                        for fc in range(2)]
                wsdt = [p0.tile([128, 2], bf16, tag=f"wsd{fc}", name=f"wsdt{fc}")
                        for fc in range(2)]
                Wkot = [p0.tile([128, OUT_F], bf16, tag=f"Wko{fc}", name=f"Wkot{fc}")
                        for fc in range(2)]
                # DMA order: first block's tensors, then what the sd-broadcast
                # chain needs (xT, wsd), then the rest
                lns = {0: stage_ln(0)}
                for fc in range(2):
                    nc.sync.dma_start(out=xT2[fc], in_=xT_d[fc * 128:(fc + 1) * 128, :])
                    nc.sync.dma_start(out=wsdt[fc], in_=wsd_d[fc * 128:(fc + 1) * 128, :])
                lns[1] = stage_ln(1)
                for fc in range(2):
                    nc.sync.dma_start(out=xTr2[fc], in_=xTr_d[fc * 128:(fc + 1) * 128, :])
                    nc.sync.dma_start(out=Wkot[fc], in_=Wko_d[fc * 128:(fc + 1) * 128, :])
                lns[2] = stage_ln(2)

                # s_dst row [1, N] and exp of it; broadcast both down partitions
                sd_row = p0.tile([1, N], bf16)
                for jc in range(8):
                    sps = ps0.tile([1, 512], f32, tag="sps", bufs=2)
                    for fc in range(2):
                        nc.tensor.matmul(sps, wsdt[fc][:, 1:2],
                                         xT2[fc][:, jc * 512:(jc + 1) * 512],
                                         start=(fc == 0), stop=(fc == 1))
                    nc.vector.tensor_copy(sd_row[0:1, jc * 512:(jc + 1) * 512], sps)
                import concourse.bass as bass_mod

                def bcast(dst, row, nm):
                    scratch = nc.dram_tensor(nm, [1, N], bf16)
                    nc.sync.dma_start(out=scratch[:], in_=row)
                    ap = bass_mod.AP(tensor=scratch[:].tensor,
                                     offset=scratch[:].offset,
                                     ap=[[0, 128]] + list(scratch[:].ap)[1:])
                    nc.gpsimd.dma_start(out=dst, in_=ap)

                bcast(sdb, sd_row, "sd_scratch")
                # exp(sd) broadcast: one ACT pass over the broadcast tile
                nc.scalar.activation(out=edb, in_=sdb, func=AF.Exp)

                # ss_col[p, ib] = s_src of own row ib*128+p; es_col = exp
                sscol_ps = ps0.tile([128, NBLK], f32, tag="sscol")
                for ib in range(NBLK):
                    for fc in range(2):
                        nc.tensor.matmul(sscol_ps[:, ib:ib + 1],
                                         xTr2[fc][:, ib * 128:(ib + 1) * 128],
                                         wsdt[fc][:, 0:1],
                                         start=(fc == 0), stop=(fc == 1))
                nc.vector.tensor_copy(ss_col, sscol_ps)
                nc.scalar.activation(out=es_col, in_=ss_col, func=AF.Exp)

                # k = x @ (W @ W_out)  (Wko folded on host; no h needed)
                ones_col = p0.tile([128, 1], bf16)
                nc.vector.memset(ones_col, 1.0)
                ksum_ps = ps0.tile([OUT_F, 1], f32, tag="ksum")
                for ib in range(NBLK):
                    kps = ps0.tile([128, OUT_F], f32, tag="kps", bufs=2)
                    for fc in range(2):
                        nc.tensor.matmul(kps,
                                         xTr2[fc][:, ib * 128:(ib + 1) * 128],
                                         Wkot[fc],
                                         start=(fc == 0), stop=(fc == 1))
                    # fold b/(aN+b) into ktil so kt1 is a pure per-row scale
                    nc.vector.tensor_scalar(out=ktil[ib], in0=kps,
                                            scalar1=float(LIN_B / RS2C),
                                            scalar2=None, op0=ALU.mult)
                    # ksum[d] += sum_i k[i, d] over this block's rows
                    nc.tensor.matmul(ksum_ps, ktil[ib], ones_col,
                                     start=(ib == 0), stop=(ib == NBLK - 1))
                # ksum was built from the pre-scaled ktil; correct for it
                nc.vector.tensor_scalar(out=C64, in0=ksum_ps,
                                        scalar1=float(LIN_A / LIN_B), scalar2=None,
                                        op0=ALU.mult)

            # ---------------- main loop ----------------
            aggpool = ctx.enter_context(tc.tile_pool(name="agg", bufs=1,
                                                     space="PSUM"))
            aggp = [aggpool.tile([OUT_F, 512], f32, tag=f"agg{j}", name=f"agg{j}")
                    for j in range(8)]
            sms = {0: stage_sm(0), 1: stage_sm(1)}
            for ib in range(NBLK):
                if ib + 3 < NBLK:
                    lns[ib + 3] = stage_ln(ib + 3)
                if ib + 2 < NBLK:
                    sms[ib + 2] = stage_sm(ib + 2)
                ad, v = lns.pop(ib)
                sm = sms.pop(ib)
                last = ib == NBLK - 1
                if ib in B_BLOCKS:
                    rs1 = rpool.tile([128, 1], f32, tag="rs1")
                    # q = 1/v (negative); M = P*adj; e1 = (M+1)*q, rowsum
                    nc.vector.reciprocal_approx_fast(out=v, in_=v)
                    nc.vector.tensor_tensor(out=sm, in0=sm, in1=ad, op=ALU.mult)
                    nc.vector.scalar_tensor_tensor(out=sm, in0=sm, scalar=1.0,
                                                   in1=v, op0=ALU.add,
                                                   op1=ALU.mult, accum_out=rs1)
                elif not last:
                    rs1 = rpool.tile([128, 1], f32, tag="rs1")
                    # g = ln(-v+eps); m = sm*adj; t = m-g; e1 = exp(t), rowsum
                    nc.scalar.activation(out=v, in_=v, func=AF.Ln, bias=epsb,
                                         scale=-1.0)
                    nc.vector.tensor_tensor(out=sm, in0=sm, in1=ad, op=ALU.mult)
                    nc.vector.tensor_tensor(out=sm, in0=sm, in1=v,
                                            op=ALU.subtract)
                    nc.scalar.activation(out=sm, in_=sm, func=AF.Exp,
                                         accum_out=rs1)
                else:
                    # final block: run the chain in 4 column chunks so the
                    # post-DMA tail is short
                    rc = [rpool.tile([128, 1], f32, tag=f"rc{c}",
                                     name=f"rc{c}") for c in range(4)]
                    for c in range(4):
                        cs = slice(c * 1024, (c + 1) * 1024)
                        nc.scalar.activation(out=v[:, cs], in_=v[:, cs],
                                             func=AF.Ln, bias=epsb, scale=-1.0)
                        nc.vector.tensor_tensor(out=sm[:, cs], in0=sm[:, cs],
                                                in1=ad[:, cs], op=ALU.mult)
                        nc.vector.tensor_tensor(out=sm[:, cs], in0=sm[:, cs],
                                                in1=v[:, cs], op=ALU.subtract)
                        nc.scalar.activation(out=sm[:, cs], in_=sm[:, cs],
                                             func=AF.Exp, accum_out=rc[c])
                    rs1 = rpool.tile([128, 1], f32, tag="rs1")
                    nc.vector.tensor_tensor(out=rc[0], in0=rc[0], in1=rc[1],
                                            op=ALU.add)
                    nc.vector.tensor_tensor(out=rc[2], in0=rc[2], in1=rc[3],
                                            op=ALU.add)
                    nc.vector.tensor_tensor(out=rs1, in0=rc[0], in1=rc[2],
                                            op=ALU.add)

                rs1r = rpool.tile([128, 1], f32, tag="rs1r")
                nc.vector.reciprocal(rs1r, rs1)
                kt1 = rpool.tile([128, OUT_F], bf16, tag="kt1")
                nc.vector.tensor_scalar(out=kt1, in0=ktil[ib], scalar1=rs1r,
                                        scalar2=None, op0=ALU.mult)

                for ns in range(8):
                    nc.tensor.matmul(aggp[ns], kt1,
                                     sm[:, ns * 512:(ns + 1) * 512],
                                     start=(ib == 0), stop=(ib == NBLK - 1))

            # ---------------- epilogue ----------------
            with tc.tile_pool(name="fin", bufs=1) as fpool:
                outT = fpool.tile([OUT_F, N], f32)
                for ns in range(8):
                    if ns % 2 == 0:
                        nc.scalar.activation(out=outT[:, ns * 512:(ns + 1) * 512],
                                             in_=aggp[ns], func=AF.Identity,
                                             bias=C64, scale=1.0)
                    else:
                        nc.vector.tensor_scalar(out=outT[:, ns * 512:(ns + 1) * 512],
                                                in0=aggp[ns], scalar1=C64,
                                                scalar2=None, op0=ALU.add)
                    if ns % 2 == 1:
                        nc.sync.dma_start(
                            out=outT_d[:, (ns - 1) * 512:(ns + 1) * 512],
                            in_=outT[:, (ns - 1) * 512:(ns + 1) * 512])

    nc.compile()
    return nc


def _get_module():
    if "nc" not in _cache:
        _cache["nc"] = _build_module()
    return _cache["nc"]


def kernel(x, adj, noise, W, a_src, a_dst, W_out):
    from concourse.bass_utils import run_bass_kernel_spmd

    nc = _get_module()

    x = np.asarray(x, dtype=np.float32)
    adj = np.asarray(adj, dtype=np.float32)
    noise = np.asarray(noise, dtype=np.float32)
    W = np.asarray(W, dtype=np.float32)
    a_src = np.asarray(a_src, dtype=np.float32)
    a_dst = np.asarray(a_dst, dtype=np.float32)
    W_out = np.asarray(W_out, dtype=np.float32)

    # fold the per-head score weights: s = (x @ W) @ a_flat / H == x @ (W @ a_flat / H)
    w_src = (W @ a_src.reshape(-1)) / H
    w_dst = (W @ a_dst.reshape(-1)) / H
    wsd = np.ascontiguousarray(np.stack([w_src, w_dst], axis=1)).astype(ml_dtypes.bfloat16)
    adj_bf = adj.astype(ml_dtypes.bfloat16)  # exact for 0/1 values
    Wko = np.ascontiguousarray(W @ W_out).astype(ml_dtypes.bfloat16)

    in_maps = []
    for core in range(N_CORES):
        b, rb = core // 2, core % 2
        rows = slice(rb * RB, (rb + 1) * RB)
        xTb = np.ascontiguousarray(x[b].T).astype(ml_dtypes.bfloat16)
        in_maps.append({
            "xT": xTb,
            "xTr": np.ascontiguousarray(xTb[:, rows]),
            "adj_s": np.ascontiguousarray(adj_bf[rows, :]),
            "noise_s": np.ascontiguousarray(noise[b, rows, :]),
            "wsd": wsd,
            "Wko": Wko,
        })

    res = run_bass_kernel_spmd(nc, in_maps, list(range(N_CORES)))
    kernel._last_results = res

    out = np.empty((B, N, OUT_F), dtype=np.float32)
    for b in range(B):
        acc = res.results[2 * b]["outT"].astype(np.float32) + \
            res.results[2 * b + 1]["outT"].astype(np.float32)
        out[b] = acc.T
    return out
